# revision 1
# baseline (speedup 1.0000x reference)
"""Trainium2 Bass kernel for nn_LocallyDense (gather -> 41 grouped GEMMs -> concat
-> Dense -> LeakyReLU), sharded over 8 NeuronCores.

Sharding: expert-parallel over groups. Each core owns 5 full groups (slots 0-4)
plus 1/8 of group 40's contraction dim (slot 5) — legal because the final
Dense is contraction-sharded and the cross-core ReduceScatter sums partial
products, so partial hT contributions for a split group sum correctly by
linearity. This gives every core exactly 10496+pad gathered rows (perfect
balance, no dummy slots) with a single SPMD NEFF.

The gather runs as SWDGE dma_gather over x^T (bf16): the int16 index limit
(D=65536 > 32767) is handled by splitting each slot's indices into lo(<32768)
/ hi(>=32768, rebased) segments, each padded to a global fixed size with dummy
index 0 whose W rows are zeroed. Phase-1 GEMMs run in bf16 (PSUM accumulates
fp32); phase 2 runs in fp32. A 512KB ReduceScatter distributes the summed
output 1/8 per core; bias+LeakyReLU run on each shard; the host concatenates.
"""

import numpy as np
import ml_dtypes

import concourse.bacc as bacc
import concourse.bass as bass
import concourse.mybir as mybir
import concourse.tile as tile
from concourse.bass_utils import run_bass_kernel_spmd

NCORES = 8
FULL_SLOTS = 5          # full groups per core
SLOTS = FULL_SLOTS + 1  # + 1 split-group slot
B, D, N, G, O, E = 256, 65536, 41, 2048, 256, 512
HALF = 32768
K2 = SLOTS * 2          # hT k-chunks per core
F32 = mybir.dt.float32
BF16 = mybir.dt.bfloat16
I16 = mybir.dt.int16
NEG_SLOPE = 0.2
BF = ml_dtypes.bfloat16


def _pad128(n):
    return -(-n // 128) * 128


def _prep_inputs(x, group_idx, W, b, W3, b3):
    """Host-side sharding/layout prep. Returns (in_maps, sizes dict)."""
    group_idx = group_idx.astype(np.int64)

    # slot assignment: core c -> groups [5c, 5c+5) + group 40 rows [256c, 256c+256)
    SPAN = G // NCORES  # 256
    lo_masks = group_idx < HALF

    S_LO = max(_pad128(int(lo_masks[n].sum())) for n in range(FULL_SLOTS * NCORES))
    S_HI = max(_pad128(G - int(lo_masks[n].sum())) for n in range(FULL_SLOTS * NCORES))
    s6lo = [int(lo_masks[40, c * SPAN : (c + 1) * SPAN].sum()) for c in range(NCORES)]
    S_LO6 = max(_pad128(v) for v in s6lo)
    S_HI6 = max(_pad128(SPAN - v) for v in s6lo)
    C = (S_LO + S_HI) // 128
    C6 = (S_LO6 + S_HI6) // 128

    xTb = np.ascontiguousarray(x.T.astype(BF))  # (D, B) bf16
    b3bc = np.ascontiguousarray(np.broadcast_to(b3, (16, E))).astype(np.float32)

    def idx_pattern(arr, S):
        """(S,) int16 -> [128, S/16] wrapped+replicated pattern."""
        pat = arr.reshape(S // 16, 16).T  # (16, S/16)
        return np.tile(pat, (8, 1))

    def split_pad(idx, S_lo, S_hi):
        """Returns (idx_lo padded, idx_hi padded, lo_positions, hi_positions)."""
        lo_pos = np.where(idx < HALF)[0]
        hi_pos = np.where(idx >= HALF)[0]
        il = np.zeros(S_lo, np.int16)
        ih = np.zeros(S_hi, np.int16)
        il[: len(lo_pos)] = idx[lo_pos].astype(np.int16)
        ih[: len(hi_pos)] = (idx[hi_pos] - HALF).astype(np.int16)
        return il, ih, lo_pos, hi_pos

    in_maps = []
    for core in range(NCORES):
        idx_lo = np.zeros((128, FULL_SLOTS, S_LO // 16), np.int16)
        idx_hi = np.zeros((128, FULL_SLOTS, S_HI // 16), np.int16)
        Wp = np.zeros((FULL_SLOTS, S_LO + S_HI, O), np.float32)
        bias = np.zeros((128, K2), np.float32)
        W3l = np.zeros((K2 * 128, E), np.float32)
        for s in range(FULL_SLOTS):
            n = core * FULL_SLOTS + s
            il, ih, lo_pos, hi_pos = split_pad(group_idx[n], S_LO, S_HI)
            idx_lo[:, s, :] = idx_pattern(il, S_LO)
            idx_hi[:, s, :] = idx_pattern(ih, S_HI)
            Wp[s, : len(lo_pos)] = W[n, lo_pos]
            Wp[s, S_LO : S_LO + len(hi_pos)] = W[n, hi_pos]
            bias[:, s * 2] = b[n, 0:128]
            bias[:, s * 2 + 1] = b[n, 128:256]
            W3l[s * 256 : (s + 1) * 256] = W3[n * 256 : (n + 1) * 256]
        # slot 5: 1/8 of group 40's contraction dim
        span = group_idx[40, core * SPAN : (core + 1) * SPAN]
        il6, ih6, lo6, hi6 = split_pad(span, S_LO6, S_HI6)
        Wp6 = np.zeros((S_LO6 + S_HI6, O), np.float32)
        Wp6[: len(lo6)] = W[40, core * SPAN + lo6]
        Wp6[S_LO6 : S_LO6 + len(hi6)] = W[40, core * SPAN + hi6]
        if core == 0:
            bias[:, 10] = b[40, 0:128]
            bias[:, 11] = b[40, 128:256]
        W3l[10 * 128 : 12 * 128] = W3[40 * 256 : 41 * 256]

        # device layouts
        Wp_dev = (
            Wp.reshape(FULL_SLOTS, C, 128, O).transpose(0, 2, 1, 3)
            .reshape(FULL_SLOTS, 128, C * O).astype(BF)
        )
        Wp6_dev = (
            Wp6.reshape(C6, 128, O).transpose(1, 0, 2).reshape(128, C6 * O).astype(BF)
        )
        W3_dev = np.ascontiguousarray(
            W3l.reshape(K2, 128, E).transpose(1, 0, 2).reshape(128, K2 * E)
        )
        pmat = np.zeros((128, 16), np.float32)
        pmat[np.arange(128), np.arange(128) % 16] = 1.0
        in_maps.append(
            {
                "pmat": pmat,
                "xTb": xTb,
                "idx_lo": np.ascontiguousarray(idx_lo),
                "idx_hi": np.ascontiguousarray(idx_hi),
                "idx_lo6": np.ascontiguousarray(idx_pattern(il6, S_LO6)),
                "idx_hi6": np.ascontiguousarray(idx_pattern(ih6, S_HI6)),
                "Wp": np.ascontiguousarray(Wp_dev),
                "Wp6": np.ascontiguousarray(Wp6_dev),
                "W3l": W3_dev,
                "bias": bias,
                "b3bc": b3bc,
            }
        )
    return in_maps, dict(S_LO=S_LO, S_HI=S_HI, S_LO6=S_LO6, S_HI6=S_HI6, C=C, C6=C6)


def _build(sz):
    S_LO, S_HI, S_LO6, S_HI6, C, C6 = (
        sz["S_LO"], sz["S_HI"], sz["S_LO6"], sz["S_HI6"], sz["C"], sz["C6"]
    )

    nc = bacc.Bacc(num_devices=NCORES)
    xT_d = nc.dram_tensor("xTb", [D, B], BF16, kind="ExternalInput")
    il_d = nc.dram_tensor("idx_lo", [128, FULL_SLOTS, S_LO // 16], I16, kind="ExternalInput")
    ih_d = nc.dram_tensor("idx_hi", [128, FULL_SLOTS, S_HI // 16], I16, kind="ExternalInput")
    il6_d = nc.dram_tensor("idx_lo6", [128, S_LO6 // 16], I16, kind="ExternalInput")
    ih6_d = nc.dram_tensor("idx_hi6", [128, S_HI6 // 16], I16, kind="ExternalInput")
    wp_d = nc.dram_tensor("Wp", [FULL_SLOTS, 128, C * O], BF16, kind="ExternalInput")
    wp6_d = nc.dram_tensor("Wp6", [128, C6 * O], BF16, kind="ExternalInput")
    w3_d = nc.dram_tensor("W3l", [128, K2 * E], F32, kind="ExternalInput")
    bias_d = nc.dram_tensor("bias", [128, K2], F32, kind="ExternalInput")
    b3_d = nc.dram_tensor("b3bc", [16, E], F32, kind="ExternalInput")
    pmat_d = nc.dram_tensor("pmat", [128, 16], F32, kind="ExternalInput")
    out_d = nc.dram_tensor("out", [16, 2, E], F32, kind="ExternalOutput")

    with tile.TileContext(nc) as tc:
        with (
            tc.tile_pool(name="const", bufs=1) as constp,
            tc.tile_pool(name="gpool", bufs=4) as gpool,
            tc.tile_pool(name="wpool", bufs=4) as wpool,
            tc.tile_pool(name="ps1", bufs=4, space="PSUM") as ps1,
            tc.tile_pool(name="ps2", bufs=1, space="PSUM") as ps2,
            tc.tile_pool(name="psr", bufs=1, space="PSUM") as psr,
            tc.tile_pool(name="dram", bufs=1, space="DRAM") as dramp,
        ):
            il_t = constp.tile([128, FULL_SLOTS, S_LO // 16], I16)
            ih_t = constp.tile([128, FULL_SLOTS, S_HI // 16], I16)
            il6_t = constp.tile([128, S_LO6 // 16], I16)
            ih6_t = constp.tile([128, S_HI6 // 16], I16)
            bias_t = constp.tile([128, K2], F32)
            b3_t = constp.tile([16, E], F32)
            w3_t = constp.tile([128, K2, E], F32)
            nc.sync.dma_start(il_t[:], il_d[:])
            nc.sync.dma_start(ih_t[:], ih_d[:])
            nc.sync.dma_start(il6_t[:], il6_d[:])
            nc.sync.dma_start(ih6_t[:], ih6_d[:])

            hT_t = constp.tile([128, K2, B], F32)

            # slot 5 (small) first so the PE gets work ~15us earlier
            slot_order = [SLOTS - 1] + list(range(FULL_SLOTS))

            # emit all gathers first so GpSimd streams them back-to-back
            gts = {}
            for s in slot_order:
                cs = C if s < FULL_SLOTS else C6
                gt = gpool.tile([128, cs, B], BF16, tag="gt" if s < FULL_SLOTS else "gt6")
                if s < FULL_SLOTS:
                    nc.gpsimd.dma_gather(
                        gt[:, 0 : S_LO // 128, :], xT_d[0:HALF, :], il_t[:, s, :],
                        S_LO, S_LO, B, single_packet=False,
                    )
                    nc.gpsimd.dma_gather(
                        gt[:, S_LO // 128 : cs, :], xT_d[HALF:D, :], ih_t[:, s, :],
                        S_HI, S_HI, B, single_packet=False,
                    )
                else:
                    nc.gpsimd.dma_gather(
                        gt[:, 0 : S_LO6 // 128, :], xT_d[0:HALF, :], il6_t[:],
                        S_LO6, S_LO6, B, single_packet=False,
                    )
                    nc.gpsimd.dma_gather(
                        gt[:, S_LO6 // 128 : cs, :], xT_d[HALF:D, :], ih6_t[:],
                        S_HI6, S_HI6, B, single_packet=False,
                    )
                wt = wpool.tile([128, cs, O], BF16, tag="wt" if s < FULL_SLOTS else "wt6")
                if s < FULL_SLOTS:
                    nc.sync.dma_start(wt[:], wp_d[s].rearrange("p (c o) -> p c o", o=O))
                else:
                    nc.sync.dma_start(wt[:], wp6_d[:].rearrange("p (c o) -> p c o", o=O))
                gts[s] = (gt, wt, cs)

            # bulk constants (W3 etc.) load after the gathers are in flight —
            # they are only needed once the first slot's GEMMs begin
            nc.sync.dma_start(bias_t[:], bias_d[:])
            nc.sync.dma_start(b3_t[:], b3_d[:])
            pmat_t = constp.tile([128, 16], F32)
            nc.sync.dma_start(pmat_t[:], pmat_d[:])
            nc.sync.dma_start(w3_t[:], w3_d[:].rearrange("p (k e) -> p k e", e=E))

            # phase-2 PSUM banks accumulate across the whole slot loop, so the
            # final Dense adds no PE tail after the last slot's phase-1 GEMM
            p2_0 = ps2.tile([128, E], F32, tag="p2_0")
            p2_1 = ps2.tile([128, E], F32, tag="p2_1")
            p2 = [p2_0, p2_1]

            def emit_phase2(si, s):
                for bh in range(2):
                    for oh in range(2):
                        kc = s * 2 + oh
                        nc.tensor.matmul(
                            p2[bh][:],
                            hT_t[:, kc, bh * 128 : (bh + 1) * 128],
                            w3_t[:, kc, :],
                            start=(si == 0 and oh == 0),
                            stop=(si == len(slot_order) - 1 and oh == 1),
                        )

            # phase-2 for slot k is emitted during slot k+1's phase-1 so the
            # PE never waits on the DVE bias-add round trip
            for si, s in enumerate(slot_order):
                gt, wt, cs = gts[s]
                for oh in range(2):
                    ps = ps1.tile([128, B], F32)
                    for cc in range(cs):
                        nc.tensor.matmul(
                            ps[:],
                            wt[:, cc, oh * 128 : (oh + 1) * 128],
                            gt[:, cc, :],
                            start=(cc == 0),
                            stop=(cc == cs - 1),
                        )
                    kc = s * 2 + oh
                    nc.vector.tensor_scalar_add(
                        hT_t[:, kc, :], ps[:], bias_t[:, kc : kc + 1]
                    )
                if si > 0:
                    emit_phase2(si - 1, slot_order[si - 1])
            emit_phase2(len(slot_order) - 1, slot_order[-1])

            part_t = constp.tile([128, 2, E], F32)
            for bh in range(2):
                nc.vector.tensor_copy(part_t[:, bh, :], p2[bh][:])

            ccin = dramp.tile([128, 2, E], F32)
            ccout = dramp.tile([16, 2, E], F32)
            nc.sync.dma_start(ccin[:], part_t[:])
            nc.gpsimd.collective_compute(
                "ReduceScatter",
                mybir.AluOpType.add,
                replica_groups=[list(range(NCORES))],
                ins=[ccin[:].opt()],
                outs=[ccout[:].opt()],
            )
            res_t = constp.tile([16, 2, E], F32)
            nc.sync.dma_start(res_t[:], ccout[:])
            z_t = constp.tile([16, 2, E], F32)
            for bh in range(2):
                nc.vector.tensor_add(z_t[:, bh, :], res_t[:, bh, :], b3_t[:])
            o_t = constp.tile([16, 2, E], F32)
            # LeakyReLU: max(0.2*z, z)
            nc.vector.scalar_tensor_tensor(
                o_t[:], z_t[:], NEG_SLOPE, z_t[:],
                op0=mybir.AluOpType.mult, op1=mybir.AluOpType.max,
            )
            nc.sync.dma_start(out_d[:], o_t[:])
    nc.compile()
    return nc


def kernel_with_results(x, group_idx, W, b, W3, b3, trace=False, warmup=True):
    in_maps, sz = _prep_inputs(
        np.asarray(x, dtype=np.float32),
        np.asarray(group_idx),
        np.asarray(W, dtype=np.float32),
        np.asarray(b, dtype=np.float32),
        np.asarray(W3, dtype=np.float32),
        np.asarray(b3, dtype=np.float32),
    )
    nc = _build(sz)
    if warmup:
        # first execute pays NEFF-load / runtime-init cross-core skew; the
        # measured run below then starts with all 8 cores aligned
        run_bass_kernel_spmd(nc, in_maps, core_ids=list(range(NCORES)))
    res = run_bass_kernel_spmd(
        nc, in_maps, core_ids=list(range(NCORES)), trace=trace
    )
    out = np.empty((B, E), np.float32)
    for c in range(NCORES):
        shard = res.results[c]["out"]  # (16, 2, E): rows 16c..16c+16 of each b-half
        out[16 * c : 16 * c + 16, :] = shard[:, 0, :]
        out[128 + 16 * c : 128 + 16 * c + 16, :] = shard[:, 1, :]
    return out, res


def kernel(**inputs):
    out, _ = kernel_with_results(**inputs)
    return out



# revision 5
# speedup vs baseline: 1.0726x; 1.0726x over previous
"""Trainium2 Bass kernel for nn_LocallyDense (gather -> 41 grouped GEMMs -> concat
-> Dense -> LeakyReLU), sharded over 8 NeuronCores.

Sharding: expert-parallel over groups. Each core owns 5 full groups (slots 0-4)
plus 1/8 of group 40's contraction dim (slot 5) — legal because the final
Dense is contraction-sharded and the cross-core ReduceScatter sums partial
products, so partial hT contributions for a split group sum correctly by
linearity. This gives every core exactly 10496 gathered rows (perfect
balance) with a single SPMD NEFF.

The gather runs as SWDGE dma_gather over x^T (bf16): the int16 index limit
(D=65536 > 32767) is handled by splitting each slot's indices into lo(<32768)
/ hi(>=32768, rebased) segments, each padded to a global fixed size with -1
(trailing negative indices are trimmed by the Q7 descriptor generator, so
padding costs no emission time; the pad rows are DVE-memset to 0 and their W
rows are zeroed). Phase-1 GEMMs run in bf16 (PSUM accumulates fp32); phase 2
also runs in bf16. A 256KB bf16 ReduceScatter distributes the summed output
1/8 per core; bias+LeakyReLU run on each shard; the host concatenates.
"""

import os

import numpy as np
import ml_dtypes

K_TRIM = os.environ.get("K_TRIM", "1") == "1"   # -1 idx padding + pad memsets
K_RSBF = os.environ.get("K_RSBF", "1") == "1"   # bf16 ReduceScatter
K_P2BF = os.environ.get("K_P2BF", "1") == "1"   # bf16 phase-2 (hT, W3)

import concourse.bacc as bacc
import concourse.bass as bass
import concourse.mybir as mybir
import concourse.tile as tile
from concourse.bass_utils import run_bass_kernel_spmd

NCORES = 8
FULL_SLOTS = 5          # full groups per core
SLOTS = FULL_SLOTS + 1  # + 1 split-group slot
B, D, N, G, O, E = 256, 65536, 41, 2048, 256, 512
HALF = 32768
K2 = SLOTS * 2          # hT k-chunks per core
F32 = mybir.dt.float32
BF16 = mybir.dt.bfloat16
I16 = mybir.dt.int16
NEG_SLOPE = 0.2
BF = ml_dtypes.bfloat16


def _pad128(n):
    return -(-n // 128) * 128


def _prep_inputs(x, group_idx, W, b, W3, b3):
    """Host-side sharding/layout prep. Returns (in_maps, sizes dict)."""
    group_idx = group_idx.astype(np.int64)

    # slot assignment: core c -> groups [5c, 5c+5) + group 40 rows [256c, 256c+256)
    SPAN = G // NCORES  # 256
    lo_masks = group_idx < HALF

    cnt_lo = np.array([int(lo_masks[n].sum()) for n in range(FULL_SLOTS * NCORES)])
    S_LO = max(_pad128(v) for v in cnt_lo)
    S_HI = max(_pad128(G - v) for v in cnt_lo)
    s6lo = np.array([int(lo_masks[40, c * SPAN : (c + 1) * SPAN].sum()) for c in range(NCORES)])
    S_LO6 = max(_pad128(v) for v in s6lo)
    S_HI6 = max(_pad128(SPAN - v) for v in s6lo)
    C = (S_LO + S_HI) // 128
    C6 = (S_LO6 + S_HI6) // 128

    # min real count per slot-position across cores -> memset start chunk
    min_lo = [int(cnt_lo.reshape(NCORES, FULL_SLOTS)[:, s].min()) for s in range(FULL_SLOTS)]
    min_hi = [int((G - cnt_lo.reshape(NCORES, FULL_SLOTS)[:, s]).min()) for s in range(FULL_SLOTS)]
    min_lo6 = int(s6lo.min())
    min_hi6 = int((SPAN - s6lo).min())

    xTb = np.ascontiguousarray(x.T.astype(BF))  # (D, B) bf16
    b3bc = np.ascontiguousarray(np.broadcast_to(b3, (16, E))).astype(np.float32)

    F_LO, F_HI, F_LO6, F_HI6 = S_LO // 16, S_HI // 16, S_LO6 // 16, S_HI6 // 16
    IDXF = FULL_SLOTS * (F_LO + F_HI) + F_LO6 + F_HI6

    def idx_pattern(arr):
        """(S,) int16 -> [128, S/16] wrapped+replicated pattern."""
        pat = arr.reshape(-1, 16).T  # (16, S/16)
        return np.tile(pat, (8, 1))

    def split_pad(idx, S_lo, S_hi):
        """Returns (idx_lo padded with -1, idx_hi padded with -1, lo_pos, hi_pos)."""
        lo_pos = np.where(idx < HALF)[0]
        hi_pos = np.where(idx >= HALF)[0]
        fill = -1 if K_TRIM else 0
        il = np.full(S_lo, fill, np.int16)
        ih = np.full(S_hi, fill, np.int16)
        il[: len(lo_pos)] = idx[lo_pos].astype(np.int16)
        ih[: len(hi_pos)] = (idx[hi_pos] - HALF).astype(np.int16)
        return il, ih, lo_pos, hi_pos

    in_maps = []
    for core in range(NCORES):
        idx_all = np.full((128, IDXF), -1, np.int16)
        Wp = np.zeros((FULL_SLOTS, S_LO + S_HI, O), np.float32)
        bias = np.zeros((128, K2), np.float32)
        W3l = np.zeros((K2 * 128, E), np.float32)
        for s in range(FULL_SLOTS):
            n = core * FULL_SLOTS + s
            il, ih, lo_pos, hi_pos = split_pad(group_idx[n], S_LO, S_HI)
            idx_all[:, s * F_LO : (s + 1) * F_LO] = idx_pattern(il)
            idx_all[
                :,
                FULL_SLOTS * F_LO + s * F_HI : FULL_SLOTS * F_LO + (s + 1) * F_HI,
            ] = idx_pattern(ih)
            Wp[s, : len(lo_pos)] = W[n, lo_pos]
            Wp[s, S_LO : S_LO + len(hi_pos)] = W[n, hi_pos]
            bias[:, s * 2] = b[n, 0:128]
            bias[:, s * 2 + 1] = b[n, 128:256]
            W3l[s * 256 : (s + 1) * 256] = W3[n * 256 : (n + 1) * 256]
        # slot 5: 1/8 of group 40's contraction dim
        span = group_idx[40, core * SPAN : (core + 1) * SPAN]
        il6, ih6, lo6, hi6 = split_pad(span, S_LO6, S_HI6)
        off6 = FULL_SLOTS * (F_LO + F_HI)
        idx_all[:, off6 : off6 + F_LO6] = idx_pattern(il6)
        idx_all[:, off6 + F_LO6 : off6 + F_LO6 + F_HI6] = idx_pattern(ih6)
        Wp6 = np.zeros((S_LO6 + S_HI6, O), np.float32)
        Wp6[: len(lo6)] = W[40, core * SPAN + lo6]
        Wp6[S_LO6 : S_LO6 + len(hi6)] = W[40, core * SPAN + hi6]
        if core == 0:
            bias[:, 10] = b[40, 0:128]
            bias[:, 11] = b[40, 128:256]
        W3l[10 * 128 : 12 * 128] = W3[40 * 256 : 41 * 256]

        # device layouts
        Wp_dev = (
            Wp.reshape(FULL_SLOTS, C, 128, O).transpose(0, 2, 1, 3)
            .reshape(FULL_SLOTS, 128, C * O).astype(BF)
        )
        Wp6_dev = (
            Wp6.reshape(C6, 128, O).transpose(1, 0, 2).reshape(128, C6 * O).astype(BF)
        )
        W3_dev = np.ascontiguousarray(
            W3l.reshape(K2, 128, E).transpose(1, 0, 2).reshape(128, K2 * E).astype(BF)
        )
        in_maps.append(
            {
                "xTb": xTb,
                "idx_all": np.ascontiguousarray(idx_all),
                "Wp": np.ascontiguousarray(Wp_dev),
                "Wp6": np.ascontiguousarray(Wp6_dev),
                "W3l": W3_dev,
                "bias": bias,
                "b3bc": b3bc,
            }
        )
    sz = dict(
        S_LO=S_LO, S_HI=S_HI, S_LO6=S_LO6, S_HI6=S_HI6, C=C, C6=C6, IDXF=IDXF,
        min_lo=min_lo, min_hi=min_hi, min_lo6=min_lo6, min_hi6=min_hi6,
    )
    return in_maps, sz


def _build(sz):
    S_LO, S_HI, S_LO6, S_HI6, C, C6, IDXF = (
        sz["S_LO"], sz["S_HI"], sz["S_LO6"], sz["S_HI6"], sz["C"], sz["C6"], sz["IDXF"]
    )
    F_LO, F_HI, F_LO6, F_HI6 = S_LO // 16, S_HI // 16, S_LO6 // 16, S_HI6 // 16
    C_LO, C_LO6 = S_LO // 128, S_LO6 // 128

    nc = bacc.Bacc(num_devices=NCORES)
    xT_d = nc.dram_tensor("xTb", [D, B], BF16, kind="ExternalInput")
    idx_d = nc.dram_tensor("idx_all", [128, IDXF], I16, kind="ExternalInput")
    wp_d = nc.dram_tensor("Wp", [FULL_SLOTS, 128, C * O], BF16, kind="ExternalInput")
    wp6_d = nc.dram_tensor("Wp6", [128, C6 * O], BF16, kind="ExternalInput")
    w3_d = nc.dram_tensor("W3l", [128, K2 * E], BF16 if K_P2BF else F32, kind="ExternalInput")
    bias_d = nc.dram_tensor("bias", [128, K2], F32, kind="ExternalInput")
    b3_d = nc.dram_tensor("b3bc", [16, E], F32, kind="ExternalInput")
    out_d = nc.dram_tensor("out", [16, 2, E], F32, kind="ExternalOutput")

    with tile.TileContext(nc) as tc:
        with (
            tc.tile_pool(name="const", bufs=1) as constp,
            tc.tile_pool(name="gpool", bufs=4) as gpool,
            tc.tile_pool(name="wpool", bufs=4) as wpool,
            tc.tile_pool(name="ps1", bufs=4, space="PSUM") as ps1,
            tc.tile_pool(name="ps2", bufs=1, space="PSUM") as ps2,
            tc.tile_pool(name="dram", bufs=1, space="DRAM") as dramp,
        ):
            idx_t = constp.tile([128, IDXF], I16)
            nc.sync.dma_start(idx_t[:], idx_d[:])

            hT_t = constp.tile([128, K2, B], BF16 if K_P2BF else F32)

            # full slots first; small slot 5 last so the post-gather tail
            # (its phase-1 + final phase-2 chunk) is minimal
            slot_order = list(range(FULL_SLOTS)) + [SLOTS - 1]

            # emit all gathers first so GpSimd streams them back-to-back
            gts = {}
            for s in slot_order:
                cs = C if s < FULL_SLOTS else C6
                gt = gpool.tile([128, cs, B], BF16, tag="gt" if s < FULL_SLOTS else "gt6")
                if s < FULL_SLOTS:
                    # zero the pad region (trimmed descriptors leave garbage)
                    if K_TRIM:
                        m0 = C_LO + sz["min_hi"][s] // 128
                        nc.vector.memset(gt[:, sz["min_lo"][s] // 128 : C_LO, :], 0.0)
                        nc.vector.memset(gt[:, m0:cs, :], 0.0)
                    nc.gpsimd.dma_gather(
                        gt[:, 0:C_LO, :], xT_d[0:HALF, :],
                        idx_t[:, s * F_LO : (s + 1) * F_LO],
                        S_LO, S_LO, B, single_packet=False,
                    )
                    nc.gpsimd.dma_gather(
                        gt[:, C_LO:cs, :], xT_d[HALF:D, :],
                        idx_t[:, FULL_SLOTS * F_LO + s * F_HI : FULL_SLOTS * F_LO + (s + 1) * F_HI],
                        S_HI, S_HI, B, single_packet=False,
                    )
                else:
                    off6 = FULL_SLOTS * (F_LO + F_HI)
                    if K_TRIM:
                        m0 = C_LO6 + sz["min_hi6"] // 128
                        nc.vector.memset(gt[:, sz["min_lo6"] // 128 : C_LO6, :], 0.0)
                        nc.vector.memset(gt[:, m0:cs, :], 0.0)
                    nc.gpsimd.dma_gather(
                        gt[:, 0:C_LO6, :], xT_d[0:HALF, :],
                        idx_t[:, off6 : off6 + F_LO6],
                        S_LO6, S_LO6, B, single_packet=False,
                    )
                    nc.gpsimd.dma_gather(
                        gt[:, C_LO6:cs, :], xT_d[HALF:D, :],
                        idx_t[:, off6 + F_LO6 : off6 + F_LO6 + F_HI6],
                        S_HI6, S_HI6, B, single_packet=False,
                    )
                wt = wpool.tile([128, cs, O], BF16, tag="wt" if s < FULL_SLOTS else "wt6")
                if s < FULL_SLOTS:
                    nc.sync.dma_start(wt[:], wp_d[s].rearrange("p (c o) -> p c o", o=O))
                else:
                    nc.sync.dma_start(wt[:], wp6_d[:].rearrange("p (c o) -> p c o", o=O))
                gts[s] = (gt, wt, cs)

            # bulk constants (W3 etc.) load after the gathers are in flight —
            # they are only needed once the first slot's GEMMs begin
            bias_t = constp.tile([128, K2], F32)
            b3_t = constp.tile([16, E], F32)
            w3_t = constp.tile([128, K2, E], BF16 if K_P2BF else F32)
            nc.sync.dma_start(bias_t[:], bias_d[:])
            nc.sync.dma_start(b3_t[:], b3_d[:])
            nc.sync.dma_start(w3_t[:], w3_d[:].rearrange("p (k e) -> p k e", e=E))

            # phase-2 PSUM banks accumulate across the whole slot loop, so the
            # final Dense adds no PE tail after the last slot's phase-1 GEMM
            p2_0 = ps2.tile([128, E], F32, tag="p2_0")
            p2_1 = ps2.tile([128, E], F32, tag="p2_1")
            p2 = [p2_0, p2_1]

            def emit_phase2(si, s):
                for bh in range(2):
                    for oh in range(2):
                        kc = s * 2 + oh
                        nc.tensor.matmul(
                            p2[bh][:],
                            hT_t[:, kc, bh * 128 : (bh + 1) * 128],
                            w3_t[:, kc, :],
                            start=(si == 0 and oh == 0),
                            stop=(si == len(slot_order) - 1 and oh == 1),
                        )

            # phase-2 for slot k is emitted during slot k+1's phase-1 so the
            # PE never waits on the DVE bias-add round trip
            for si, s in enumerate(slot_order):
                gt, wt, cs = gts[s]
                for oh in range(2):
                    ps = ps1.tile([128, B], F32)
                    for cc in range(cs):
                        nc.tensor.matmul(
                            ps[:],
                            wt[:, cc, oh * 128 : (oh + 1) * 128],
                            gt[:, cc, :],
                            start=(cc == 0),
                            stop=(cc == cs - 1),
                        )
                    kc = s * 2 + oh
                    nc.vector.tensor_scalar_add(
                        hT_t[:, kc, :], ps[:], bias_t[:, kc : kc + 1]
                    )
                if si > 0:
                    emit_phase2(si - 1, slot_order[si - 1])
            emit_phase2(len(slot_order) - 1, slot_order[-1])

            part_t = constp.tile([128, 2, E], BF16 if K_RSBF else F32)
            for bh in range(2):
                nc.vector.tensor_copy(part_t[:, bh, :], p2[bh][:])

            ccin = dramp.tile([128, 2, E], BF16 if K_RSBF else F32)
            ccout = dramp.tile([16, 2, E], BF16 if K_RSBF else F32)
            nc.sync.dma_start(ccin[:], part_t[:])
            nc.gpsimd.collective_compute(
                "ReduceScatter",
                mybir.AluOpType.add,
                replica_groups=[list(range(NCORES))],
                ins=[ccin[:].opt()],
                outs=[ccout[:].opt()],
            )
            res_t = constp.tile([16, 2, E], BF16 if K_RSBF else F32)
            nc.sync.dma_start(res_t[:], ccout[:])
            z_t = constp.tile([16, 2, E], F32)
            for bh in range(2):
                nc.vector.tensor_add(z_t[:, bh, :], res_t[:, bh, :], b3_t[:])
            o_t = constp.tile([16, 2, E], F32)
            # LeakyReLU: max(0.2*z, z)
            nc.vector.scalar_tensor_tensor(
                o_t[:], z_t[:], NEG_SLOPE, z_t[:],
                op0=mybir.AluOpType.mult, op1=mybir.AluOpType.max,
            )
            nc.sync.dma_start(out_d[:], o_t[:])
    nc.compile()
    return nc


def kernel_with_results(x, group_idx, W, b, W3, b3, trace=False, warmup=True):
    in_maps, sz = _prep_inputs(
        np.asarray(x, dtype=np.float32),
        np.asarray(group_idx),
        np.asarray(W, dtype=np.float32),
        np.asarray(b, dtype=np.float32),
        np.asarray(W3, dtype=np.float32),
        np.asarray(b3, dtype=np.float32),
    )
    nc = _build(sz)
    if warmup:
        # first execute pays NEFF-load / runtime-init cross-core skew; the
        # measured run below then starts with all 8 cores aligned
        run_bass_kernel_spmd(nc, in_maps, core_ids=list(range(NCORES)))
    res = run_bass_kernel_spmd(
        nc, in_maps, core_ids=list(range(NCORES)), trace=trace
    )
    out = np.empty((B, E), np.float32)
    for c in range(NCORES):
        shard = res.results[c]["out"]  # (16, 2, E): rows 16c..16c+16 of each b-half
        out[16 * c : 16 * c + 16, :] = shard[:, 0, :]
        out[128 + 16 * c : 128 + 16 * c + 16, :] = shard[:, 1, :]
    return out, res


def kernel(**inputs):
    out, _ = kernel_with_results(**inputs)
    return out
